# revision 20
# baseline (speedup 1.0000x reference)
# Multi-head attention (B=4, S=2048, D=1024, H=16) on 8 TRN2 NeuronCores.
#
# Sharding: 8 cores = 4 batches x 2 query-halves. Each core:
#   - projects K/V for its batch's full sequence (duplicated across the 2
#     cores that share a batch -- cheaper than any collective),
#   - projects Q for its 1024 query rows,
#   - runs all 16 heads of attention for those query rows,
#   - computes the final output projection for its rows.
# Host-side work is only slicing/transposing/bf16-casting inputs and
# concatenating the 8 output slices -- no reductions happen on the host.
#
# Kernel-internal choices:
#   - activation inputs are pre-transposed on host to [D, rows] so every
#     matmul contracts over the partition dim,
#   - bf16 operands with fp32 PSUM accumulation (halves SBUF + DMA and
#     enables fast weight loads); the softmax denominator reciprocal path
#     stays fp32/f32r,
#   - Q/K/V projections are SBUF-resident (no DRAM roundtrips); only the
#     per-head attention output bounces through DRAM for the final matmul,
#   - softmax skips the max-subtraction (scores ~ N(0,1) by construction;
#     the 1/sqrt(dk) scale is folded into wq on the host),
#   - the softmax denominator comes from a ones-column appended to each
#     V tile, so it lands in the same PSUM tile as the attn@V output,
#   - scores for a head pair run on PE row-strips (partitions 0-63/64-127)
#     so the two dk=64 matmuls overlap on the systolic array,
#   - phase order QP -> VP -> KP: attention consumes per-tile deps on the
#     resident K tiles, so it overlaps the tail of the K projection.

import numpy as np

B, S, D, H, DK = 4, 2048, 1024, 16, 64
P = 128
NCORES = 8

TRACE = False  # set by test.py to capture an NTFF profile
LAST_RESULTS = {}  # test.py reads exec_time_ns etc. from here


class Cfg:
    def __init__(self, D_, S_, SQ, H_, FS):
        assert D_ == H_ * DK
        self.D, self.S, self.SQ, self.H, self.FS = D_, S_, SQ, H_, FS
        self.XS = S_ // 2          # x-stage slice width (half of kv seq)
        self.ND = D_ // P          # d_model partition tiles
        self.NKT = D_ // P         # contraction tiles over d_model
        self.NQF = SQ // FS        # query free-dim slices
        self.NSP = S_ // P         # key partition tiles
        self.NHP = H_ // 2         # head pairs
        self.NDF = D_ // FS        # d_model free-dim slices
        self.HPF = FS // DK        # heads per FS slice
        assert self.XS % FS == 0 and self.XS % P == 0
        assert SQ % P == 0 and SQ <= self.XS * 2
        assert H_ % 2 == 0 and FS % DK == 0


FULL_CFG_ARGS = (D, S, S // 2, H, 512)


def build_nc(cfg: Cfg):
    import concourse.mybir as mybir
    import concourse.tile as tile
    from concourse import bacc
    from contextlib import ExitStack

    f32 = mybir.dt.float32
    f32r = mybir.dt.float32r
    bf = mybir.dt.bfloat16

    D_, S_, SQ, FS, XS = cfg.D, cfg.S, cfg.SQ, cfg.FS, cfg.XS
    VW = DK + 1  # v-tile width incl. ones column

    nc = bacc.Bacc("TRN2", debug=False, num_devices=NCORES)

    xqT = nc.dram_tensor("xqT", [D_, SQ], bf, kind="ExternalInput").ap()
    xkT = nc.dram_tensor("xkT", [D_, S_], bf, kind="ExternalInput").ap()
    xvT = nc.dram_tensor("xvT", [D_, S_], bf, kind="ExternalInput").ap()
    wq = nc.dram_tensor("wq", [D_, D_], bf, kind="ExternalInput").ap()
    wk = nc.dram_tensor("wk", [D_, D_], bf, kind="ExternalInput").ap()
    wv = nc.dram_tensor("wv", [D_, D_], bf, kind="ExternalInput").ap()
    wo = nc.dram_tensor("wo", [D_, D_], bf, kind="ExternalInput").ap()
    bq = nc.dram_tensor("bq", [1, D_], bf, kind="ExternalInput").ap()
    bk = nc.dram_tensor("bk", [1, D_], bf, kind="ExternalInput").ap()
    bv = nc.dram_tensor("bv", [1, D_], bf, kind="ExternalInput").ap()
    bo = nc.dram_tensor("bo", [1, D_], bf, kind="ExternalInput").ap()
    onesb = nc.dram_tensor("onesb", [P, FS], bf, kind="ExternalInput").ap()
    out = nc.dram_tensor("out", [SQ, D_], f32, kind="ExternalOutput").ap()

    with tile.TileContext(nc) as tc, ExitStack() as ctx:
        pool = lambda name, bufs, space=None: ctx.enter_context(
            tc.tile_pool(name=name, bufs=bufs, **({"space": space} if space else {}))
        )
        kptp = pool("kpt", cfg.ND)     # resident K^T  [P, S] bf16
        qptp = pool("qpt", cfg.ND)     # resident Q^T  [P, SQ] bf16
        vprp = pool("vpr", cfg.NSP)    # resident V    [P, H*VW] bf16
        xst = pool("xst", 16)
        xqp = pool("xqp", 8)
        wsm = pool("wsm", 16)
        wbg = pool("wbg", 8)
        pex = pool("pex", 6)
        otc = pool("otc", 3)
        recp = pool("rec", 2)
        posp = pool("pos", 3)
        d64p = pool("d64", 2)
        cps = pool("cps", 3)
        cst = pool("cst", 1)
        psA = pool("psA", 2, "PSUM")
        psO = pool("psO", 4, "PSUM")
        dram = pool("dram", 1, "DRAM")

        ot_d = dram.tile([D_, SQ], bf, name="ot_d", tag="ot_d")

        # --- constants ---
        bias_sb = {}
        for name, ap in (("bq", bq), ("bk", bk), ("bv", bv), ("bo", bo)):
            t = cst.tile([1, D_], bf, name=name, tag=name)
            nc.sync.dma_start(t[:, :], ap[:, :])
            bias_sb[name] = t
        ones = cst.tile([P, FS], bf, name="ones", tag="ones")
        nc.sync.dma_start(ones[:, :], onesb[:, :])

        # resident tiles
        qpt_t = [qptp.tile([P, SQ], bf, name='qptr', tag='qptr') for _ in range(cfg.ND)]
        kpt_t = [kptp.tile([P, S_], bf, name='kpt', tag='kpt') for _ in range(cfg.ND)]
        vpr_t = [vprp.tile([P, cfg.H * VW], bf, name='vpr', tag='vpr')
                 for _ in range(cfg.NSP)]

        # --- Q projection: QPT[d_out, rq] = (xq @ wq + bq)^T, resident ---
        xq_t = []
        for kt in range(cfg.NKT):
            t = xqp.tile([P, SQ], bf, name='xqp', tag='xqp')
            nc.sync.dma_start(t[:, :], xqT[kt * P:(kt + 1) * P, :])
            xq_t.append(t)
        for dt_ in range(cfg.ND):
            ps = psA.tile([P, cfg.NQF * FS], f32, name='psa', tag='psa')
            for kt in range(cfg.NKT):
                wt = wsm.tile([P, P], bf, name='wsm', tag='wsm')
                nc.sync.dma_start(wt[:, :], wq[kt * P:(kt + 1) * P, dt_ * P:(dt_ + 1) * P])
                for qf in range(cfg.NQF):
                    nc.tensor.matmul(
                        ps[:, qf * FS:(qf + 1) * FS], wt[:, :],
                        xq_t[kt][:, qf * FS:(qf + 1) * FS],
                        start=(kt == 0), stop=False)
            for qf in range(cfg.NQF):
                nc.tensor.matmul(
                    ps[:, qf * FS:(qf + 1) * FS],
                    bias_sb["bq"][0:1, dt_ * P:(dt_ + 1) * P],
                    ones[0:1, :], start=False, stop=True)
            nc.vector.tensor_copy(qpt_t[dt_][:, :], ps[:, 0:SQ])

        # --- V projection: VP[rk, dv] head-major, resident w/ ones columns ---
        xv_t = []
        for kt in range(cfg.NKT):
            for half in range(2):
                t = xst.tile([P, XS], bf, name='xst', tag='xst')
                nc.sync.dma_start(t[:, :], xvT[kt * P:(kt + 1) * P, half * XS:(half + 1) * XS])
                xv_t.append(t)  # index 2*kt+half
        for df in range(cfg.NDF):
            wv_t = []
            for kt in range(cfg.NKT):
                t = wbg.tile([P, FS], bf, name='wbg', tag='wbg')
                nc.sync.dma_start(t[:, :], wv[kt * P:(kt + 1) * P, df * FS:(df + 1) * FS])
                wv_t.append(t)
            for rk in range(cfg.NSP):
                half, rt = rk // (XS // P), rk % (XS // P)
                ps = psA.tile([P, FS], f32, name='psa', tag='psa')
                for kt in range(cfg.NKT):
                    nc.tensor.matmul(
                        ps[:, :], xv_t[2 * kt + half][:, rt * P:(rt + 1) * P],
                        wv_t[kt][:, :], start=(kt == 0), stop=False)
                nc.tensor.matmul(
                    ps[:, :], ones[0:1, 0:P],
                    bias_sb["bv"][0:1, df * FS:(df + 1) * FS],
                    start=False, stop=True)
                h0 = df * cfg.HPF
                dst = vpr_t[rk].rearrange("p (h c) -> p h c", c=VW)
                nc.vector.tensor_copy(
                    dst[:, h0:h0 + cfg.HPF, 0:DK],
                    ps.rearrange("p (h v) -> p h v", v=DK))
        for rk in range(cfg.NSP):
            dst = vpr_t[rk].rearrange("p (h c) -> p h c", c=VW)
            nc.vector.memset(dst[:, :, DK:DK + 1], 1.0)

        # --- K projection interleaved with attention: KP(dt) feeds ATT(hp=dt),
        # so PE-heavy projection work fills the ACT-bound attention slack ---
        xk_t = []
        for kt in range(cfg.NKT):
            for half in range(2):
                t = xst.tile([P, XS], bf, name='xst', tag='xst')
                nc.sync.dma_start(t[:, :], xkT[kt * P:(kt + 1) * P, half * XS:(half + 1) * XS])
                xk_t.append(t)  # index 2*kt+half

        def kp_block(dt_):
            for sfp in range(S_ // (2 * FS)):
                ps = psA.tile([P, 2 * FS], f32, name='psa', tag='psa')
                for kt in range(cfg.NKT):
                    wt = wsm.tile([P, P], bf, name='wsm', tag='wsm')
                    nc.sync.dma_start(wt[:, :], wk[kt * P:(kt + 1) * P, dt_ * P:(dt_ + 1) * P])
                    for g in range(2):
                        sfg = 2 * sfp + g
                        half, sf = sfg // (XS // FS), sfg % (XS // FS)
                        nc.tensor.matmul(
                            ps[:, g * FS:(g + 1) * FS], wt[:, :],
                            xk_t[2 * kt + half][:, sf * FS:(sf + 1) * FS],
                            start=(kt == 0), stop=False)
                for g in range(2):
                    nc.tensor.matmul(
                        ps[:, g * FS:(g + 1) * FS],
                        bias_sb["bk"][0:1, dt_ * P:(dt_ + 1) * P],
                        ones[0:1, :], start=False, stop=True)
                nc.vector.tensor_copy(
                    kpt_t[dt_][:, 2 * sfp * FS:(2 * sfp + 2) * FS], ps[:, :])

        def att_block(hp):
            # attention for this head pair on PE row-strips; both query
            # slices interleaved so PSUM accumulation never drains the pipe
            h0, h1 = 2 * hp, 2 * hp + 1
            po = {(qf, hh): psO.tile([DK + 1, FS], f32, name='pso', tag='pso')
                  for qf in range(cfg.NQF) for hh in (h0, h1)}
            for rt in range(cfg.NSP):
                for qf in range(cfg.NQF):
                    tsp = psA.tile([P, 2 * FS], f32, name='psa', tag='psa')
                    nc.tensor.matmul(
                        tsp[:, 0:FS], kpt_t[hp][0:DK, rt * P:(rt + 1) * P],
                        qpt_t[hp][0:DK, qf * FS:(qf + 1) * FS], start=True, stop=True)
                    nc.tensor.matmul(
                        tsp[:, FS:2 * FS], kpt_t[hp][DK:P, rt * P:(rt + 1) * P],
                        qpt_t[hp][DK:P, qf * FS:(qf + 1) * FS], start=True, stop=True)
                    pp = pex.tile([P, 2 * FS], bf, name='pex', tag='pex')
                    nc.scalar.activation(pp[:, :], tsp[:, :],
                                         mybir.ActivationFunctionType.Exp)
                    nc.tensor.matmul(
                        po[(qf, h0)][:, :], vpr_t[rt][:, h0 * VW:h0 * VW + VW],
                        pp[:, 0:FS],
                        start=(rt == 0), stop=(rt == cfg.NSP - 1))
                    nc.tensor.matmul(
                        po[(qf, h1)][:, :], vpr_t[rt][:, h1 * VW:h1 * VW + VW],
                        pp[:, FS:2 * FS],
                        start=(rt == 0), stop=(rt == cfg.NSP - 1))
            for qf in range(cfg.NQF):
                for hh in (h0, h1):
                    # free the PSUM bank fast, normalize SBUF-side
                    pos = posp.tile([DK + 1, FS], f32, name='pos', tag='pos')
                    nc.vector.tensor_copy(pos[:, :], po[(qf, hh)][:, :])
                    rc = recp.tile([DK + 1, FS], f32, name='rec', tag='rec')
                    nc.vector.reciprocal(rc[DK:DK + 1, :], pos[DK:DK + 1, :])
                    # broadcast the recip row across DK partitions via a
                    # DRAM bounce (stride-0 partition read)
                    den_d = dram.tile([1, FS], f32, name='den_d', tag='den_d', bufs=4)
                    nc.gpsimd.dma_start(den_d[:, :], rc[DK:DK + 1, :])
                    d6 = d64p.tile([DK, FS], f32, name='d64', tag='d64')
                    nc.gpsimd.dma_start(d6[:, :], den_d.broadcast_to([DK, FS]))
                    ot = otc.tile([DK, FS], bf, name='otc', tag='otc')
                    nc.vector.tensor_mul(ot[:, :], pos[0:DK, :], d6[:, :])
                    nc.gpsimd.dma_start(
                        ot_d[hh * DK:(hh + 1) * DK, qf * FS:(qf + 1) * FS], ot[:, :])

        for hp in range(cfg.NHP):
            kp_block(hp)
            att_block(hp)

        # --- output projection: out[rq, df] = O @ wo + bo ---
        for df in range(cfg.NDF):
            wo_t = []
            for kt in range(cfg.NKT):
                t = wbg.tile([P, FS], bf, name='wbg', tag='wbg')
                nc.sync.dma_start(t[:, :], wo[kt * P:(kt + 1) * P, df * FS:(df + 1) * FS])
                wo_t.append(t)
            for rqt in range(SQ // P):
                ps = psA.tile([P, FS], f32, name='psa', tag='psa')
                for kt in range(cfg.NKT):
                    ot_t = wsm.tile([P, P], bf, name='wsm', tag='wsm')
                    nc.sync.dma_start(ot_t[:, :], ot_d[kt * P:(kt + 1) * P, rqt * P:(rqt + 1) * P])
                    nc.tensor.matmul(ps[:, :], ot_t[:, :], wo_t[kt][:, :],
                                     start=(kt == 0), stop=False)
                nc.tensor.matmul(
                    ps[:, :], ones[0:1, 0:P],
                    bias_sb["bo"][0:1, df * FS:(df + 1) * FS],
                    start=False, stop=True)
                ct = cps.tile([P, FS], f32, name='cps', tag='cps')
                nc.vector.tensor_copy(ct[:, :], ps[:, :])
                nc.scalar.dma_start(out[rqt * P:(rqt + 1) * P, df * FS:(df + 1) * FS], ct[:, :])

    nc.compile()
    return nc


def shard_inputs(q, k, v, wq, bq, wk, bk, wv, bv, wo, bo, sq, fs):
    """Build the 8 per-core input maps (bf16 operands) for the full problem."""
    import ml_dtypes
    bf = ml_dtypes.bfloat16
    scale = np.float32(1.0 / np.sqrt(DK))

    def c(x, dt=bf):
        return np.ascontiguousarray(np.asarray(x, np.float32).astype(dt))

    common = {
        "wq": c(np.asarray(wq, np.float32) * scale),
        "wk": c(wk), "wv": c(wv), "wo": c(wo),
        "bq": c((np.asarray(bq, np.float32) * scale).reshape(1, -1)),
        "bk": c(np.asarray(bk, np.float32).reshape(1, -1)),
        "bv": c(np.asarray(bv, np.float32).reshape(1, -1)),
        "bo": c(np.asarray(bo, np.float32).reshape(1, -1)),
        "onesb": np.ones((P, fs), bf),
    }
    q = np.asarray(q, np.float32)
    k = np.asarray(k, np.float32)
    v = np.asarray(v, np.float32)
    in_maps = []
    for core in range(NCORES):
        b, hf = core // 2, core % 2
        m = dict(common)
        m["xqT"] = c(q[b, hf * sq:(hf + 1) * sq, :].T)
        m["xkT"] = c(k[b].T)
        m["xvT"] = c(v[b].T)
        in_maps.append(m)
    return in_maps


_cached = {}


def kernel(q, k, v, wq, bq, wk, bk, wv, bv, wo, bo):
    from concourse import bass_utils

    global LAST_RESULTS
    cfg = Cfg(*FULL_CFG_ARGS)
    if "nc" not in _cached:
        _cached["nc"] = build_nc(cfg)
    nc = _cached["nc"]
    in_maps = shard_inputs(q, k, v, wq, bq, wk, bk, wv, bv, wo, bo, cfg.SQ, cfg.FS)
    res = bass_utils.run_bass_kernel_spmd(
        nc, in_maps, core_ids=list(range(NCORES)), trace=TRACE)
    LAST_RESULTS["res"] = res
    out = np.empty((B, S, D), np.float32)
    for core in range(NCORES):
        b, hf = core // 2, core % 2
        out[b, hf * cfg.SQ:(hf + 1) * cfg.SQ, :] = res.results[core]["out"]
    return out


# revision 21
# speedup vs baseline: 1.1537x; 1.1537x over previous
# Multi-head attention (B=4, S=2048, D=1024, H=16) on 8 TRN2 NeuronCores.
#
# Sharding: 8 cores = 4 batches x 2 query-halves. Each core:
#   - projects K/V for its batch's full sequence (duplicated across the 2
#     cores that share a batch -- cheaper than any collective),
#   - projects Q for its 1024 query rows,
#   - runs all 16 heads of attention for those query rows,
#   - computes the final output projection for its rows.
# Host-side work is only slicing/transposing/bf16-casting inputs and
# concatenating the 8 output slices -- no reductions happen on the host.
#
# Kernel-internal choices:
#   - activation inputs are pre-transposed on host to [D, rows] so every
#     matmul contracts over the partition dim,
#   - bf16 operands with fp32 PSUM accumulation (halves SBUF + DMA and
#     enables fast weight loads); the softmax denominator reciprocal path
#     stays fp32/f32r,
#   - Q/K/V projections are SBUF-resident (no DRAM roundtrips); only the
#     per-head attention output bounces through DRAM for the final matmul,
#   - softmax skips the max-subtraction (scores ~ N(0,1) by construction;
#     the 1/sqrt(dk) scale is folded into wq on the host),
#   - the softmax denominator comes from a ones-column appended to each
#     V tile, so it lands in the same PSUM tile as the attn@V output,
#   - scores for a head pair run on PE row-strips (partitions 0-63/64-127)
#     so the two dk=64 matmuls overlap on the systolic array,
#   - phase order QP -> VP -> KP: attention consumes per-tile deps on the
#     resident K tiles, so it overlaps the tail of the K projection.

import numpy as np

B, S, D, H, DK = 4, 2048, 1024, 16, 64
P = 128
NCORES = 8

TRACE = False  # set by test.py to capture an NTFF profile
LAST_RESULTS = {}  # test.py reads exec_time_ns etc. from here


class Cfg:
    def __init__(self, D_, S_, SQ, H_, FS):
        assert D_ == H_ * DK
        self.D, self.S, self.SQ, self.H, self.FS = D_, S_, SQ, H_, FS
        self.XS = S_ // 2          # x-stage slice width (half of kv seq)
        self.ND = D_ // P          # d_model partition tiles
        self.NKT = D_ // P         # contraction tiles over d_model
        self.NQF = SQ // FS        # query free-dim slices
        self.NSP = S_ // P         # key partition tiles
        self.NHP = H_ // 2         # head pairs
        self.NDF = D_ // FS        # d_model free-dim slices
        self.HPF = FS // DK        # heads per FS slice
        assert self.XS % FS == 0 and self.XS % P == 0
        assert SQ % P == 0 and SQ <= self.XS * 2
        assert H_ % 2 == 0 and FS % DK == 0


FULL_CFG_ARGS = (D, S, S // 2, H, 512)


def build_nc(cfg: Cfg):
    import concourse.mybir as mybir
    import concourse.tile as tile
    from concourse import bacc
    from contextlib import ExitStack

    f32 = mybir.dt.float32
    f32r = mybir.dt.float32r
    bf = mybir.dt.bfloat16

    D_, S_, SQ, FS, XS = cfg.D, cfg.S, cfg.SQ, cfg.FS, cfg.XS
    VW = DK + 1  # v-tile width incl. ones column

    nc = bacc.Bacc("TRN2", debug=False, num_devices=NCORES)

    xqT = nc.dram_tensor("xqT", [D_, SQ], bf, kind="ExternalInput").ap()
    xkT = nc.dram_tensor("xkT", [D_, S_], bf, kind="ExternalInput").ap()
    xvT = nc.dram_tensor("xvT", [D_, S_], bf, kind="ExternalInput").ap()
    wq = nc.dram_tensor("wq", [D_, D_], bf, kind="ExternalInput").ap()
    wk = nc.dram_tensor("wk", [D_, D_], bf, kind="ExternalInput").ap()
    wv = nc.dram_tensor("wv", [D_, D_], bf, kind="ExternalInput").ap()
    wo = nc.dram_tensor("wo", [D_, D_], bf, kind="ExternalInput").ap()
    bq = nc.dram_tensor("bq", [1, D_], bf, kind="ExternalInput").ap()
    bk = nc.dram_tensor("bk", [1, D_], bf, kind="ExternalInput").ap()
    bv = nc.dram_tensor("bv", [1, D_], bf, kind="ExternalInput").ap()
    bo = nc.dram_tensor("bo", [1, D_], bf, kind="ExternalInput").ap()
    onesb = nc.dram_tensor("onesb", [P, FS], bf, kind="ExternalInput").ap()
    out = nc.dram_tensor("out", [SQ, D_], f32, kind="ExternalOutput").ap()

    with tile.TileContext(nc) as tc, ExitStack() as ctx:
        pool = lambda name, bufs, space=None: ctx.enter_context(
            tc.tile_pool(name=name, bufs=bufs, **({"space": space} if space else {}))
        )
        kptp = pool("kpt", cfg.ND)     # resident K^T  [P, S] bf16
        qptp = pool("qpt", cfg.ND)     # resident Q^T  [P, SQ] bf16
        vprp = pool("vpr", cfg.NSP)    # resident V    [P, H*VW] bf16
        xst = pool("xst", 16)
        xqp = pool("xqp", 8)
        wsm = pool("wsm", 16)
        wbg = pool("wbg", 8)
        pex = pool("pex", 6)
        otc = pool("otc", 3)
        recp = pool("rec", 2)
        posp = pool("pos", 5)
        d64p = pool("d64", 2)
        cps = pool("cps", 3)
        cst = pool("cst", 1)
        psA = pool("psA", 2, "PSUM")
        psO = pool("psO", 4, "PSUM")
        dram = pool("dram", 1, "DRAM")

        ot_d = dram.tile([D_, SQ], bf, name="ot_d", tag="ot_d")

        # --- constants ---
        bias_sb = {}
        for name, ap in (("bq", bq), ("bk", bk), ("bv", bv), ("bo", bo)):
            t = cst.tile([1, D_], bf, name=name, tag=name)
            nc.sync.dma_start(t[:, :], ap[:, :])
            bias_sb[name] = t
        ones = cst.tile([P, FS], bf, name="ones", tag="ones")
        nc.sync.dma_start(ones[:, :], onesb[:, :])

        # resident tiles
        qpt_t = [qptp.tile([P, SQ], bf, name='qptr', tag='qptr') for _ in range(cfg.ND)]
        kpt_t = [kptp.tile([P, S_], bf, name='kpt', tag='kpt') for _ in range(cfg.ND)]
        vpr_t = [vprp.tile([P, cfg.H * VW], bf, name='vpr', tag='vpr')
                 for _ in range(cfg.NSP)]

        # --- Q projection: QPT[d_out, rq] = (xq @ wq + bq)^T, resident ---
        xq_t = []
        for kt in range(cfg.NKT):
            t = xqp.tile([P, SQ], bf, name='xqp', tag='xqp')
            nc.sync.dma_start(t[:, :], xqT[kt * P:(kt + 1) * P, :])
            xq_t.append(t)
        for dt_ in range(cfg.ND):
            ps = psA.tile([P, cfg.NQF * FS], f32, name='psa', tag='psa')
            for kt in range(cfg.NKT):
                wt = wsm.tile([P, P], bf, name='wsm', tag='wsm')
                nc.sync.dma_start(wt[:, :], wq[kt * P:(kt + 1) * P, dt_ * P:(dt_ + 1) * P])
                for qf in range(cfg.NQF):
                    nc.tensor.matmul(
                        ps[:, qf * FS:(qf + 1) * FS], wt[:, :],
                        xq_t[kt][:, qf * FS:(qf + 1) * FS],
                        start=(kt == 0), stop=False)
            for qf in range(cfg.NQF):
                nc.tensor.matmul(
                    ps[:, qf * FS:(qf + 1) * FS],
                    bias_sb["bq"][0:1, dt_ * P:(dt_ + 1) * P],
                    ones[0:1, :], start=False, stop=True)
            nc.vector.tensor_copy(qpt_t[dt_][:, :], ps[:, 0:SQ])

        # --- V projection: VP[rk, dv] head-major, resident w/ ones columns ---
        xv_t = []
        for kt in range(cfg.NKT):
            for half in range(2):
                t = xst.tile([P, XS], bf, name='xst', tag='xst')
                nc.sync.dma_start(t[:, :], xvT[kt * P:(kt + 1) * P, half * XS:(half + 1) * XS])
                xv_t.append(t)  # index 2*kt+half
        for df in range(cfg.NDF):
            wv_t = []
            for kt in range(cfg.NKT):
                t = wbg.tile([P, FS], bf, name='wbg', tag='wbg')
                nc.sync.dma_start(t[:, :], wv[kt * P:(kt + 1) * P, df * FS:(df + 1) * FS])
                wv_t.append(t)
            for rk in range(cfg.NSP):
                half, rt = rk // (XS // P), rk % (XS // P)
                ps = psA.tile([P, FS], f32, name='psa', tag='psa')
                for kt in range(cfg.NKT):
                    nc.tensor.matmul(
                        ps[:, :], xv_t[2 * kt + half][:, rt * P:(rt + 1) * P],
                        wv_t[kt][:, :], start=(kt == 0), stop=False)
                nc.tensor.matmul(
                    ps[:, :], ones[0:1, 0:P],
                    bias_sb["bv"][0:1, df * FS:(df + 1) * FS],
                    start=False, stop=True)
                h0 = df * cfg.HPF
                dst = vpr_t[rk].rearrange("p (h c) -> p h c", c=VW)
                nc.vector.tensor_copy(
                    dst[:, h0:h0 + cfg.HPF, 0:DK],
                    ps.rearrange("p (h v) -> p h v", v=DK))
        for rk in range(cfg.NSP):
            dst = vpr_t[rk].rearrange("p (h c) -> p h c", c=VW)
            nc.vector.memset(dst[:, :, DK:DK + 1], 1.0)

        # --- K projection interleaved with attention: KP(dt) feeds ATT(hp=dt),
        # so PE-heavy projection work fills the ACT-bound attention slack ---
        xk_t = []
        for kt in range(cfg.NKT):
            for half in range(2):
                t = xst.tile([P, XS], bf, name='xst', tag='xst')
                nc.sync.dma_start(t[:, :], xkT[kt * P:(kt + 1) * P, half * XS:(half + 1) * XS])
                xk_t.append(t)  # index 2*kt+half

        def kp_block(dt_):
            for sfp in range(S_ // (2 * FS)):
                ps = psA.tile([P, 2 * FS], f32, name='psa', tag='psa')
                for kt in range(cfg.NKT):
                    wt = wsm.tile([P, P], bf, name='wsm', tag='wsm')
                    nc.sync.dma_start(wt[:, :], wk[kt * P:(kt + 1) * P, dt_ * P:(dt_ + 1) * P])
                    for g in range(2):
                        sfg = 2 * sfp + g
                        half, sf = sfg // (XS // FS), sfg % (XS // FS)
                        nc.tensor.matmul(
                            ps[:, g * FS:(g + 1) * FS], wt[:, :],
                            xk_t[2 * kt + half][:, sf * FS:(sf + 1) * FS],
                            start=(kt == 0), stop=False)
                for g in range(2):
                    nc.tensor.matmul(
                        ps[:, g * FS:(g + 1) * FS],
                        bias_sb["bk"][0:1, dt_ * P:(dt_ + 1) * P],
                        ones[0:1, :], start=False, stop=True)
                nc.vector.tensor_copy(
                    kpt_t[dt_][:, 2 * sfp * FS:(2 * sfp + 2) * FS], ps[:, :])

        def att_block(hp):
            # attention for this head pair on PE row-strips; both query
            # slices interleaved so PSUM accumulation never drains the pipe
            h0, h1 = 2 * hp, 2 * hp + 1
            po = {(qf, hh): psO.tile([DK + 1, FS], f32, name='pso', tag='pso')
                  for qf in range(cfg.NQF) for hh in (h0, h1)}
            for rt in range(cfg.NSP):
                for qf in range(cfg.NQF):
                    tsp = psA.tile([P, 2 * FS], f32, name='psa', tag='psa')
                    nc.tensor.matmul(
                        tsp[:, 0:FS], kpt_t[hp][0:DK, rt * P:(rt + 1) * P],
                        qpt_t[hp][0:DK, qf * FS:(qf + 1) * FS], start=True, stop=True)
                    nc.tensor.matmul(
                        tsp[:, FS:2 * FS], kpt_t[hp][DK:P, rt * P:(rt + 1) * P],
                        qpt_t[hp][DK:P, qf * FS:(qf + 1) * FS], start=True, stop=True)
                    pp = pex.tile([P, 2 * FS], bf, name='pex', tag='pex')
                    nc.scalar.activation(pp[:, :], tsp[:, :],
                                         mybir.ActivationFunctionType.Exp)
                    nc.tensor.matmul(
                        po[(qf, h0)][:, :], vpr_t[rt][:, h0 * VW:h0 * VW + VW],
                        pp[:, 0:FS],
                        start=(rt == 0), stop=(rt == cfg.NSP - 1))
                    nc.tensor.matmul(
                        po[(qf, h1)][:, :], vpr_t[rt][:, h1 * VW:h1 * VW + VW],
                        pp[:, FS:2 * FS],
                        start=(rt == 0), stop=(rt == cfg.NSP - 1))
            poss = {}
            for qf in range(cfg.NQF):
                for hh in (h0, h1):
                    # free the PSUM banks fast (before any slow reciprocal)
                    pos = posp.tile([DK + 1, FS], f32, name='pos', tag='pos')
                    nc.vector.tensor_copy(pos[:, :], po[(qf, hh)][:, :])
                    poss[(qf, hh)] = pos
            for qf in range(cfg.NQF):
                for hh in (h0, h1):
                    pos = poss[(qf, hh)]
                    rc = recp.tile([DK + 1, FS], f32, name='rec', tag='rec')
                    nc.vector.reciprocal(rc[DK:DK + 1, :], pos[DK:DK + 1, :])
                    # broadcast the recip row across DK partitions via a
                    # DRAM bounce (stride-0 partition read)
                    den_d = dram.tile([1, FS], f32, name='den_d', tag='den_d', bufs=4)
                    nc.sync.dma_start(den_d[:, :], rc[DK:DK + 1, :])
                    d6 = d64p.tile([DK, FS], f32, name='d64', tag='d64')
                    nc.gpsimd.dma_start(d6[:, :], den_d.broadcast_to([DK, FS]))
                    ot = otc.tile([DK, FS], bf, name='otc', tag='otc')
                    nc.vector.tensor_mul(ot[:, :], pos[0:DK, :], d6[:, :])
                    nc.sync.dma_start(
                        ot_d[hh * DK:(hh + 1) * DK, qf * FS:(qf + 1) * FS], ot[:, :])

        for hp in range(cfg.NHP):
            kp_block(hp)
            att_block(hp)

        # --- output projection: out[rq, df] = O @ wo + bo ---
        for df in range(cfg.NDF):
            wo_t = []
            for kt in range(cfg.NKT):
                t = wbg.tile([P, FS], bf, name='wbg', tag='wbg')
                nc.sync.dma_start(t[:, :], wo[kt * P:(kt + 1) * P, df * FS:(df + 1) * FS])
                wo_t.append(t)
            for rqt in range(SQ // P):
                ps = psA.tile([P, FS], f32, name='psa', tag='psa')
                for kt in range(cfg.NKT):
                    ot_t = wsm.tile([P, P], bf, name='wsm', tag='wsm')
                    nc.sync.dma_start(ot_t[:, :], ot_d[kt * P:(kt + 1) * P, rqt * P:(rqt + 1) * P])
                    nc.tensor.matmul(ps[:, :], ot_t[:, :], wo_t[kt][:, :],
                                     start=(kt == 0), stop=False)
                nc.tensor.matmul(
                    ps[:, :], ones[0:1, 0:P],
                    bias_sb["bo"][0:1, df * FS:(df + 1) * FS],
                    start=False, stop=True)
                ct = cps.tile([P, FS], f32, name='cps', tag='cps')
                nc.vector.tensor_copy(ct[:, :], ps[:, :])
                nc.sync.dma_start(out[rqt * P:(rqt + 1) * P, df * FS:(df + 1) * FS], ct[:, :])

    nc.compile()
    return nc


def shard_inputs(q, k, v, wq, bq, wk, bk, wv, bv, wo, bo, sq, fs):
    """Build the 8 per-core input maps (bf16 operands) for the full problem."""
    import ml_dtypes
    bf = ml_dtypes.bfloat16
    scale = np.float32(1.0 / np.sqrt(DK))

    def c(x, dt=bf):
        return np.ascontiguousarray(np.asarray(x, np.float32).astype(dt))

    common = {
        "wq": c(np.asarray(wq, np.float32) * scale),
        "wk": c(wk), "wv": c(wv), "wo": c(wo),
        "bq": c((np.asarray(bq, np.float32) * scale).reshape(1, -1)),
        "bk": c(np.asarray(bk, np.float32).reshape(1, -1)),
        "bv": c(np.asarray(bv, np.float32).reshape(1, -1)),
        "bo": c(np.asarray(bo, np.float32).reshape(1, -1)),
        "onesb": np.ones((P, fs), bf),
    }
    q = np.asarray(q, np.float32)
    k = np.asarray(k, np.float32)
    v = np.asarray(v, np.float32)
    in_maps = []
    for core in range(NCORES):
        b, hf = core // 2, core % 2
        m = dict(common)
        m["xqT"] = c(q[b, hf * sq:(hf + 1) * sq, :].T)
        m["xkT"] = c(k[b].T)
        m["xvT"] = c(v[b].T)
        in_maps.append(m)
    return in_maps


_cached = {}


def kernel(q, k, v, wq, bq, wk, bk, wv, bv, wo, bo):
    from concourse import bass_utils

    global LAST_RESULTS
    cfg = Cfg(*FULL_CFG_ARGS)
    if "nc" not in _cached:
        _cached["nc"] = build_nc(cfg)
    nc = _cached["nc"]
    in_maps = shard_inputs(q, k, v, wq, bq, wk, bk, wv, bv, wo, bo, cfg.SQ, cfg.FS)
    res = bass_utils.run_bass_kernel_spmd(
        nc, in_maps, core_ids=list(range(NCORES)), trace=TRACE)
    LAST_RESULTS["res"] = res
    out = np.empty((B, S, D), np.float32)
    for core in range(NCORES):
        b, hf = core // 2, core % 2
        out[b, hf * cfg.SQ:(hf + 1) * cfg.SQ, :] = res.results[core]["out"]
    return out


# revision 22
# speedup vs baseline: 1.1626x; 1.0077x over previous
# Multi-head attention (B=4, S=2048, D=1024, H=16) on 8 TRN2 NeuronCores.
#
# Sharding: 8 cores = 4 batches x 2 query-halves. Each core:
#   - projects K/V for its batch's full sequence (duplicated across the 2
#     cores that share a batch -- cheaper than any collective),
#   - projects Q for its 1024 query rows,
#   - runs all 16 heads of attention for those query rows,
#   - computes the final output projection for its rows.
# Host-side work is only slicing/transposing/bf16-casting inputs and
# concatenating the 8 output slices -- no reductions happen on the host.
#
# Kernel-internal choices:
#   - activation inputs are pre-transposed on host to [D, rows] so every
#     matmul contracts over the partition dim,
#   - bf16 operands with fp32 PSUM accumulation (halves SBUF + DMA and
#     enables fast weight loads); the softmax denominator reciprocal path
#     stays fp32/f32r,
#   - Q/K/V projections are SBUF-resident (no DRAM roundtrips); only the
#     per-head attention output bounces through DRAM for the final matmul,
#   - softmax skips the max-subtraction (scores ~ N(0,1) by construction;
#     the 1/sqrt(dk) scale is folded into wq on the host),
#   - the softmax denominator comes from a ones-column appended to each
#     V tile, so it lands in the same PSUM tile as the attn@V output,
#   - scores for a head pair run on PE row-strips (partitions 0-63/64-127)
#     so the two dk=64 matmuls overlap on the systolic array,
#   - phase order QP -> VP -> KP: attention consumes per-tile deps on the
#     resident K tiles, so it overlaps the tail of the K projection.

import numpy as np

B, S, D, H, DK = 4, 2048, 1024, 16, 64
P = 128
NCORES = 8

TRACE = False  # set by test.py to capture an NTFF profile
LAST_RESULTS = {}  # test.py reads exec_time_ns etc. from here


class Cfg:
    def __init__(self, D_, S_, SQ, H_, FS):
        assert D_ == H_ * DK
        self.D, self.S, self.SQ, self.H, self.FS = D_, S_, SQ, H_, FS
        self.XS = S_ // 2          # x-stage slice width (half of kv seq)
        self.ND = D_ // P          # d_model partition tiles
        self.NKT = D_ // P         # contraction tiles over d_model
        self.NQF = SQ // FS        # query free-dim slices
        self.NSP = S_ // P         # key partition tiles
        self.NHP = H_ // 2         # head pairs
        self.NDF = D_ // FS        # d_model free-dim slices
        self.HPF = FS // DK        # heads per FS slice
        assert self.XS % FS == 0 and self.XS % P == 0
        assert SQ % P == 0 and SQ <= self.XS * 2
        assert H_ % 2 == 0 and FS % DK == 0


FULL_CFG_ARGS = (D, S, S // 2, H, 512)


def build_nc(cfg: Cfg):
    import concourse.mybir as mybir
    import concourse.tile as tile
    from concourse import bacc
    from contextlib import ExitStack

    f32 = mybir.dt.float32
    f32r = mybir.dt.float32r
    bf = mybir.dt.bfloat16

    D_, S_, SQ, FS, XS = cfg.D, cfg.S, cfg.SQ, cfg.FS, cfg.XS
    VW = DK + 1  # v-tile width incl. ones column

    nc = bacc.Bacc("TRN2", debug=False, num_devices=NCORES)

    xqT = nc.dram_tensor("xqT", [D_, SQ], bf, kind="ExternalInput").ap()
    xkT = nc.dram_tensor("xkT", [D_, S_], bf, kind="ExternalInput").ap()
    xvT = nc.dram_tensor("xvT", [D_, S_], bf, kind="ExternalInput").ap()
    wq = nc.dram_tensor("wq", [D_, D_], bf, kind="ExternalInput").ap()
    wk = nc.dram_tensor("wk", [D_, D_], bf, kind="ExternalInput").ap()
    wv = nc.dram_tensor("wv", [D_, D_], bf, kind="ExternalInput").ap()
    wo = nc.dram_tensor("wo", [D_, D_], bf, kind="ExternalInput").ap()
    bq = nc.dram_tensor("bq", [1, D_], bf, kind="ExternalInput").ap()
    bk = nc.dram_tensor("bk", [1, D_], bf, kind="ExternalInput").ap()
    bv = nc.dram_tensor("bv", [1, D_], bf, kind="ExternalInput").ap()
    bo = nc.dram_tensor("bo", [1, D_], bf, kind="ExternalInput").ap()
    onesb = nc.dram_tensor("onesb", [P, FS], bf, kind="ExternalInput").ap()
    out = nc.dram_tensor("out", [SQ, D_], f32, kind="ExternalOutput").ap()

    with tile.TileContext(nc) as tc, ExitStack() as ctx:
        pool = lambda name, bufs, space=None: ctx.enter_context(
            tc.tile_pool(name=name, bufs=bufs, **({"space": space} if space else {}))
        )
        kptp = pool("kpt", cfg.ND)     # resident K^T  [P, S] bf16
        qptp = pool("qpt", cfg.ND)     # resident Q^T  [P, SQ] bf16
        vprp = pool("vpr", cfg.NSP)    # resident V    [P, H*VW] bf16
        xst = pool("xst", 16)
        xqp = pool("xqp", 8)
        wsm = pool("wsm", 20)
        wbg = pool("wbg", 8)
        pex = pool("pex", 10)
        otc = pool("otc", 3)
        recp = pool("rec", 2)
        posp = pool("pos", 5)
        d64p = pool("d64", 2)
        cps = pool("cps", 3)
        cst = pool("cst", 1)
        psA = pool("psA", 2, "PSUM")
        psO = pool("psO", 4, "PSUM")
        dram = pool("dram", 1, "DRAM")

        ot_d = [dram.tile([D_, FS], bf, name="ot_d", tag=f"ot_d{i}")
                for i in range(cfg.NQF)]

        # --- constants ---
        bias_sb = {}
        for name, ap in (("bq", bq), ("bk", bk), ("bv", bv), ("bo", bo)):
            t = cst.tile([1, D_], bf, name=name, tag=name)
            nc.sync.dma_start(t[:, :], ap[:, :])
            bias_sb[name] = t
        ones = cst.tile([P, FS], bf, name="ones", tag="ones")
        nc.sync.dma_start(ones[:, :], onesb[:, :])

        # resident tiles
        qpt_t = [qptp.tile([P, SQ], bf, name='qptr', tag='qptr') for _ in range(cfg.ND)]
        kpt_t = [kptp.tile([P, S_], bf, name='kpt', tag='kpt') for _ in range(cfg.ND)]
        vpr_t = [vprp.tile([P, cfg.H * VW], bf, name='vpr', tag='vpr')
                 for _ in range(cfg.NSP)]

        # --- Q projection: QPT[d_out, rq] = (xq @ wq + bq)^T, resident ---
        xq_t = []
        for kt in range(cfg.NKT):
            t = xqp.tile([P, SQ], bf, name='xqp', tag='xqp')
            nc.sync.dma_start(t[:, :], xqT[kt * P:(kt + 1) * P, :])
            xq_t.append(t)
        for dt_ in range(cfg.ND):
            ps = psA.tile([P, cfg.NQF * FS], f32, name='psa', tag='psa')
            for kt in range(cfg.NKT):
                wt = wsm.tile([P, P], bf, name='wsm', tag='wsm')
                nc.sync.dma_start(wt[:, :], wq[kt * P:(kt + 1) * P, dt_ * P:(dt_ + 1) * P])
                for qf in range(cfg.NQF):
                    nc.tensor.matmul(
                        ps[:, qf * FS:(qf + 1) * FS], wt[:, :],
                        xq_t[kt][:, qf * FS:(qf + 1) * FS],
                        start=(kt == 0), stop=False)
            for qf in range(cfg.NQF):
                nc.tensor.matmul(
                    ps[:, qf * FS:(qf + 1) * FS],
                    bias_sb["bq"][0:1, dt_ * P:(dt_ + 1) * P],
                    ones[0:1, :], start=False, stop=True)
            nc.vector.tensor_copy(qpt_t[dt_][:, :], ps[:, 0:SQ])

        # --- V projection: VP[rk, dv] head-major, resident w/ ones columns ---
        xv_t = []
        for kt in range(cfg.NKT):
            for half in range(2):
                t = xst.tile([P, XS], bf, name='xst', tag='xst')
                nc.sync.dma_start(t[:, :], xvT[kt * P:(kt + 1) * P, half * XS:(half + 1) * XS])
                xv_t.append(t)  # index 2*kt+half
        for df in range(cfg.NDF):
            wv_t = []
            for kt in range(cfg.NKT):
                t = wbg.tile([P, FS], bf, name='wbg', tag='wbg')
                nc.sync.dma_start(t[:, :], wv[kt * P:(kt + 1) * P, df * FS:(df + 1) * FS])
                wv_t.append(t)
            for rk in range(cfg.NSP):
                half, rt = rk // (XS // P), rk % (XS // P)
                ps = psA.tile([P, FS], f32, name='psa', tag='psa')
                for kt in range(cfg.NKT):
                    nc.tensor.matmul(
                        ps[:, :], xv_t[2 * kt + half][:, rt * P:(rt + 1) * P],
                        wv_t[kt][:, :], start=(kt == 0), stop=False)
                nc.tensor.matmul(
                    ps[:, :], ones[0:1, 0:P],
                    bias_sb["bv"][0:1, df * FS:(df + 1) * FS],
                    start=False, stop=True)
                h0 = df * cfg.HPF
                dst = vpr_t[rk].rearrange("p (h c) -> p h c", c=VW)
                nc.vector.tensor_copy(
                    dst[:, h0:h0 + cfg.HPF, 0:DK],
                    ps.rearrange("p (h v) -> p h v", v=DK))
        for rk in range(cfg.NSP):
            dst = vpr_t[rk].rearrange("p (h c) -> p h c", c=VW)
            nc.vector.memset(dst[:, :, DK:DK + 1], 1.0)

        # --- K projection interleaved with attention: KP(dt) feeds ATT(hp=dt),
        # so PE-heavy projection work fills the ACT-bound attention slack ---
        xk_t = []
        for kt in range(cfg.NKT):
            for half in range(2):
                t = xst.tile([P, XS], bf, name='xst', tag='xst')
                nc.sync.dma_start(t[:, :], xkT[kt * P:(kt + 1) * P, half * XS:(half + 1) * XS])
                xk_t.append(t)  # index 2*kt+half

        def kp_block(dt_):
            for sfp in range(S_ // (2 * FS)):
                ps = psA.tile([P, 2 * FS], f32, name='psa', tag='psa')
                for kt in range(cfg.NKT):
                    wt = wsm.tile([P, P], bf, name='wsm', tag='wsm')
                    nc.sync.dma_start(wt[:, :], wk[kt * P:(kt + 1) * P, dt_ * P:(dt_ + 1) * P])
                    for g in range(2):
                        sfg = 2 * sfp + g
                        half, sf = sfg // (XS // FS), sfg % (XS // FS)
                        nc.tensor.matmul(
                            ps[:, g * FS:(g + 1) * FS], wt[:, :],
                            xk_t[2 * kt + half][:, sf * FS:(sf + 1) * FS],
                            start=(kt == 0), stop=False)
                for g in range(2):
                    nc.tensor.matmul(
                        ps[:, g * FS:(g + 1) * FS],
                        bias_sb["bk"][0:1, dt_ * P:(dt_ + 1) * P],
                        ones[0:1, :], start=False, stop=True)
                nc.vector.tensor_copy(
                    kpt_t[dt_][:, 2 * sfp * FS:(2 * sfp + 2) * FS], ps[:, :])

        def att_block(hp):
            # attention for this head pair on PE row-strips; both query
            # slices interleaved so PSUM accumulation never drains the pipe
            h0, h1 = 2 * hp, 2 * hp + 1
            po = {(qf, hh): psO.tile([DK + 1, FS], f32, name='pso', tag='pso')
                  for qf in range(cfg.NQF) for hh in (h0, h1)}
            for rt in range(cfg.NSP):
                for qf in range(cfg.NQF):
                    tsp = psA.tile([P, 2 * FS], f32, name='psa', tag='psa')
                    nc.tensor.matmul(
                        tsp[:, 0:FS], kpt_t[hp][0:DK, rt * P:(rt + 1) * P],
                        qpt_t[hp][0:DK, qf * FS:(qf + 1) * FS], start=True, stop=True)
                    nc.tensor.matmul(
                        tsp[:, FS:2 * FS], kpt_t[hp][DK:P, rt * P:(rt + 1) * P],
                        qpt_t[hp][DK:P, qf * FS:(qf + 1) * FS], start=True, stop=True)
                    pp = pex.tile([P, 2 * FS], bf, name='pex', tag='pex')
                    nc.scalar.activation(pp[:, :], tsp[:, :],
                                         mybir.ActivationFunctionType.Exp)
                    nc.tensor.matmul(
                        po[(qf, h0)][:, :], vpr_t[rt][:, h0 * VW:h0 * VW + VW],
                        pp[:, 0:FS],
                        start=(rt == 0), stop=(rt == cfg.NSP - 1))
                    nc.tensor.matmul(
                        po[(qf, h1)][:, :], vpr_t[rt][:, h1 * VW:h1 * VW + VW],
                        pp[:, FS:2 * FS],
                        start=(rt == 0), stop=(rt == cfg.NSP - 1))
            poss = {}
            for qf in range(cfg.NQF):
                for hh in (h0, h1):
                    # free the PSUM banks fast (before any slow reciprocal)
                    pos = posp.tile([DK + 1, FS], f32, name='pos', tag='pos')
                    nc.vector.tensor_copy(pos[:, :], po[(qf, hh)][:, :])
                    poss[(qf, hh)] = pos
            for qf in range(cfg.NQF):
                for hh in (h0, h1):
                    pos = poss[(qf, hh)]
                    rc = recp.tile([DK + 1, FS], f32, name='rec', tag='rec')
                    nc.vector.reciprocal(rc[DK:DK + 1, :], pos[DK:DK + 1, :])
                    # broadcast the recip row across DK partitions via a
                    # DRAM bounce (stride-0 partition read)
                    den_d = dram.tile([1, FS], f32, name='den_d', tag='den_d', bufs=4)
                    nc.sync.dma_start(den_d[:, :], rc[DK:DK + 1, :])
                    d6 = d64p.tile([DK, FS], f32, name='d64', tag='d64')
                    nc.gpsimd.dma_start(d6[:, :], den_d.broadcast_to([DK, FS]))
                    ot = otc.tile([DK, FS], bf, name='otc', tag='otc')
                    nc.vector.tensor_mul(ot[:, :], pos[0:DK, :], d6[:, :])
                    nc.sync.dma_start(
                        ot_d[qf][hh * DK:(hh + 1) * DK, :], ot[:, :])

        for hp in range(cfg.NHP):
            kp_block(hp)
            att_block(hp)

        # --- output projection: out[rq, df] = O @ wo + bo ---
        for df in range(cfg.NDF):
            wo_t = []
            for kt in range(cfg.NKT):
                t = wbg.tile([P, FS], bf, name='wbg', tag='wbg')
                nc.sync.dma_start(t[:, :], wo[kt * P:(kt + 1) * P, df * FS:(df + 1) * FS])
                wo_t.append(t)
            for rqt in range(SQ // P):
                ps = psA.tile([P, FS], f32, name='psa', tag='psa')
                for kt in range(cfg.NKT):
                    ot_t = wsm.tile([P, P], bf, name='wsm', tag='wsm')
                    rqf, rql = divmod(rqt, FS // P)
                    nc.sync.dma_start(ot_t[:, :], ot_d[rqf][kt * P:(kt + 1) * P, rql * P:(rql + 1) * P])
                    nc.tensor.matmul(ps[:, :], ot_t[:, :], wo_t[kt][:, :],
                                     start=(kt == 0), stop=False)
                nc.tensor.matmul(
                    ps[:, :], ones[0:1, 0:P],
                    bias_sb["bo"][0:1, df * FS:(df + 1) * FS],
                    start=False, stop=True)
                ct = cps.tile([P, FS], f32, name='cps', tag='cps')
                nc.vector.tensor_copy(ct[:, :], ps[:, :])
                nc.sync.dma_start(out[rqt * P:(rqt + 1) * P, df * FS:(df + 1) * FS], ct[:, :])

    nc.compile()
    return nc


def shard_inputs(q, k, v, wq, bq, wk, bk, wv, bv, wo, bo, sq, fs):
    """Build the 8 per-core input maps (bf16 operands) for the full problem."""
    import ml_dtypes
    bf = ml_dtypes.bfloat16
    scale = np.float32(1.0 / np.sqrt(DK))

    def c(x, dt=bf):
        return np.ascontiguousarray(np.asarray(x, np.float32).astype(dt))

    common = {
        "wq": c(np.asarray(wq, np.float32) * scale),
        "wk": c(wk), "wv": c(wv), "wo": c(wo),
        "bq": c((np.asarray(bq, np.float32) * scale).reshape(1, -1)),
        "bk": c(np.asarray(bk, np.float32).reshape(1, -1)),
        "bv": c(np.asarray(bv, np.float32).reshape(1, -1)),
        "bo": c(np.asarray(bo, np.float32).reshape(1, -1)),
        "onesb": np.ones((P, fs), bf),
    }
    q = np.asarray(q, np.float32)
    k = np.asarray(k, np.float32)
    v = np.asarray(v, np.float32)
    in_maps = []
    for core in range(NCORES):
        b, hf = core // 2, core % 2
        m = dict(common)
        m["xqT"] = c(q[b, hf * sq:(hf + 1) * sq, :].T)
        m["xkT"] = c(k[b].T)
        m["xvT"] = c(v[b].T)
        in_maps.append(m)
    return in_maps


_cached = {}


def kernel(q, k, v, wq, bq, wk, bk, wv, bv, wo, bo):
    from concourse import bass_utils

    global LAST_RESULTS
    cfg = Cfg(*FULL_CFG_ARGS)
    if "nc" not in _cached:
        _cached["nc"] = build_nc(cfg)
    nc = _cached["nc"]
    in_maps = shard_inputs(q, k, v, wq, bq, wk, bk, wv, bv, wo, bo, cfg.SQ, cfg.FS)
    res = bass_utils.run_bass_kernel_spmd(
        nc, in_maps, core_ids=list(range(NCORES)), trace=TRACE)
    LAST_RESULTS["res"] = res
    out = np.empty((B, S, D), np.float32)
    for core in range(NCORES):
        b, hf = core // 2, core % 2
        out[b, hf * cfg.SQ:(hf + 1) * cfg.SQ, :] = res.results[core]["out"]
    return out


# revision 23
# speedup vs baseline: 1.2663x; 1.0892x over previous
# Multi-head attention (B=4, S=2048, D=1024, H=16) on 8 TRN2 NeuronCores.
#
# Sharding: 8 cores = 4 batches x 2 query-halves. Each core:
#   - projects K/V for its batch's full sequence (duplicated across the 2
#     cores that share a batch -- cheaper than any collective),
#   - projects Q for its 1024 query rows,
#   - runs all 16 heads of attention for those query rows,
#   - computes the final output projection for its rows.
# Host-side work is only slicing/transposing/bf16-casting inputs and
# concatenating the 8 output slices -- no reductions happen on the host.
#
# Kernel-internal choices:
#   - activation inputs are pre-transposed on host to [D, rows] so every
#     matmul contracts over the partition dim,
#   - bf16 operands with fp32 PSUM accumulation (halves SBUF + DMA and
#     enables fast weight loads); the softmax denominator reciprocal path
#     stays fp32/f32r,
#   - Q/K/V projections are SBUF-resident (no DRAM roundtrips); only the
#     per-head attention output bounces through DRAM for the final matmul,
#   - softmax skips the max-subtraction (scores ~ N(0,1) by construction;
#     the 1/sqrt(dk) scale is folded into wq on the host),
#   - the softmax denominator comes from a ones-column appended to each
#     V tile, so it lands in the same PSUM tile as the attn@V output,
#   - scores for a head pair run on PE row-strips (partitions 0-63/64-127)
#     so the two dk=64 matmuls overlap on the systolic array,
#   - phase order QP -> VP -> KP: attention consumes per-tile deps on the
#     resident K tiles, so it overlaps the tail of the K projection.

import numpy as np

B, S, D, H, DK = 4, 2048, 1024, 16, 64
P = 128
NCORES = 8

TRACE = False  # set by test.py to capture an NTFF profile
LAST_RESULTS = {}  # test.py reads exec_time_ns etc. from here


class Cfg:
    def __init__(self, D_, S_, SQ, H_, FS):
        assert D_ == H_ * DK
        self.D, self.S, self.SQ, self.H, self.FS = D_, S_, SQ, H_, FS
        self.XS = S_ // 2          # x-stage slice width (half of kv seq)
        self.ND = D_ // P          # d_model partition tiles
        self.NKT = D_ // P         # contraction tiles over d_model
        self.NQF = SQ // FS        # query free-dim slices
        self.NSP = S_ // P         # key partition tiles
        self.NHP = H_ // 2         # head pairs
        self.NDF = D_ // FS        # d_model free-dim slices
        self.HPF = FS // DK        # heads per FS slice
        assert self.XS % FS == 0 and self.XS % P == 0
        assert SQ % P == 0 and SQ <= self.XS * 2
        assert H_ % 2 == 0 and FS % DK == 0


FULL_CFG_ARGS = (D, S, S // 2, H, 512)


def build_nc(cfg: Cfg):
    import concourse.mybir as mybir
    import concourse.tile as tile
    from concourse import bacc
    from contextlib import ExitStack

    f32 = mybir.dt.float32
    f32r = mybir.dt.float32r
    bf = mybir.dt.bfloat16

    D_, S_, SQ, FS, XS = cfg.D, cfg.S, cfg.SQ, cfg.FS, cfg.XS
    VW = DK + 1  # v-tile width incl. ones column

    nc = bacc.Bacc("TRN2", debug=False, num_devices=NCORES)

    xqT = nc.dram_tensor("xqT", [D_, SQ], bf, kind="ExternalInput").ap()
    xkT = nc.dram_tensor("xkT", [D_, S_], bf, kind="ExternalInput").ap()
    xvT = nc.dram_tensor("xvT", [D_, S_], bf, kind="ExternalInput").ap()
    wq = nc.dram_tensor("wq", [D_, D_], bf, kind="ExternalInput").ap()
    wk = nc.dram_tensor("wk", [D_, D_], bf, kind="ExternalInput").ap()
    wv = nc.dram_tensor("wv", [D_, D_], bf, kind="ExternalInput").ap()
    wo = nc.dram_tensor("wo", [D_, D_], bf, kind="ExternalInput").ap()
    bq = nc.dram_tensor("bq", [1, D_], bf, kind="ExternalInput").ap()
    bk = nc.dram_tensor("bk", [1, D_], bf, kind="ExternalInput").ap()
    bv = nc.dram_tensor("bv", [1, D_], bf, kind="ExternalInput").ap()
    bo = nc.dram_tensor("bo", [1, D_], bf, kind="ExternalInput").ap()
    onesb = nc.dram_tensor("onesb", [P, FS], bf, kind="ExternalInput").ap()
    out = nc.dram_tensor("out", [SQ, D_], f32, kind="ExternalOutput").ap()

    with tile.TileContext(nc) as tc, ExitStack() as ctx:
        pool = lambda name, bufs, space=None: ctx.enter_context(
            tc.tile_pool(name=name, bufs=bufs, **({"space": space} if space else {}))
        )
        kptp = pool("kpt", cfg.ND)     # resident K^T  [P, S] bf16
        qptp = pool("qpt", cfg.ND)     # resident Q^T  [P, SQ] bf16
        vprp = pool("vpr", cfg.NSP)    # resident V    [P, H*VW] bf16
        xst = pool("xst", 16)
        xqp = pool("xqp", 8)
        wsm = pool("wsm", 20)
        wbg = pool("wbg", 8)
        pex = pool("pex", 10)
        otc = pool("otc", 3)
        recp = pool("rec", 2)
        posp = pool("pos", 5)
        d64p = pool("d64", 2)
        cps = pool("cps", 3)
        cst = pool("cst", 1)
        psA = pool("psA", 2, "PSUM")
        psO = pool("psO", 4, "PSUM")
        dram = pool("dram", 1, "DRAM")

        ot_d = [dram.tile([D_, FS], bf, name="ot_d", tag=f"ot_d{i}")
                for i in range(cfg.NQF)]

        # --- constants ---
        bias_sb = {}
        for name, ap in (("bq", bq), ("bk", bk), ("bv", bv), ("bo", bo)):
            t = cst.tile([1, D_], bf, name=name, tag=name)
            nc.sync.dma_start(t[:, :], ap[:, :])
            bias_sb[name] = t
        ones = cst.tile([P, FS], bf, name="ones", tag="ones")
        nc.sync.dma_start(ones[:, :], onesb[:, :])

        # resident tiles
        qpt_t = [qptp.tile([P, SQ], bf, name='qptr', tag='qptr') for _ in range(cfg.ND)]
        kpt_t = [kptp.tile([P, S_], bf, name='kpt', tag='kpt') for _ in range(cfg.ND)]
        vpr_t = [vprp.tile([P, cfg.H * VW], bf, name='vpr', tag='vpr')
                 for _ in range(cfg.NSP)]

        # --- Q projection: QPT[d_out, rq] = (xq @ wq + bq)^T, resident ---
        xq_t = []
        for kt in range(cfg.NKT):
            t = xqp.tile([P, SQ], bf, name='xqp', tag='xqp')
            nc.sync.dma_start(t[:, :], xqT[kt * P:(kt + 1) * P, :])
            xq_t.append(t)
        xv_t = []
        for kt in range(cfg.NKT):
            for half in range(2):
                t = xst.tile([P, XS], bf, name='xst', tag='xst')
                nc.sync.dma_start(t[:, :], xvT[kt * P:(kt + 1) * P, half * XS:(half + 1) * XS])
                xv_t.append(t)  # index 2*kt+half
        for dt_ in range(cfg.ND):
            ps = psA.tile([P, cfg.NQF * FS], f32, name='psa', tag='psa')
            for kt in range(cfg.NKT):
                wt = wsm.tile([P, P], bf, name='wsm', tag='wsm')
                nc.sync.dma_start(wt[:, :], wq[kt * P:(kt + 1) * P, dt_ * P:(dt_ + 1) * P])
                for qf in range(cfg.NQF):
                    nc.tensor.matmul(
                        ps[:, qf * FS:(qf + 1) * FS], wt[:, :],
                        xq_t[kt][:, qf * FS:(qf + 1) * FS],
                        start=(kt == 0), stop=False)
            for qf in range(cfg.NQF):
                nc.tensor.matmul(
                    ps[:, qf * FS:(qf + 1) * FS],
                    bias_sb["bq"][0:1, dt_ * P:(dt_ + 1) * P],
                    ones[0:1, :], start=False, stop=True)
            nc.vector.tensor_copy(qpt_t[dt_][:, :], ps[:, 0:SQ])

        # --- V projection: VP[rk, dv] head-major, resident w/ ones columns ---
        for df in range(cfg.NDF):
            wv_t = []
            for kt in range(cfg.NKT):
                t = wbg.tile([P, FS], bf, name='wbg', tag='wbg')
                nc.sync.dma_start(t[:, :], wv[kt * P:(kt + 1) * P, df * FS:(df + 1) * FS])
                wv_t.append(t)
            for rk in range(cfg.NSP):
                half, rt = rk // (XS // P), rk % (XS // P)
                ps = psA.tile([P, FS], f32, name='psa', tag='psa')
                for kt in range(cfg.NKT):
                    nc.tensor.matmul(
                        ps[:, :], xv_t[2 * kt + half][:, rt * P:(rt + 1) * P],
                        wv_t[kt][:, :], start=(kt == 0), stop=False)
                nc.tensor.matmul(
                    ps[:, :], ones[0:1, 0:P],
                    bias_sb["bv"][0:1, df * FS:(df + 1) * FS],
                    start=False, stop=True)
                h0 = df * cfg.HPF
                dst = vpr_t[rk].rearrange("p (h c) -> p h c", c=VW)
                nc.vector.tensor_copy(
                    dst[:, h0:h0 + cfg.HPF, 0:DK],
                    ps.rearrange("p (h v) -> p h v", v=DK))
        for rk in range(cfg.NSP):
            dst = vpr_t[rk].rearrange("p (h c) -> p h c", c=VW)
            nc.vector.memset(dst[:, :, DK:DK + 1], 1.0)

        # --- K projection interleaved with attention: KP(dt) feeds ATT(hp=dt),
        # so PE-heavy projection work fills the ACT-bound attention slack ---
        xk_t = []
        for kt in range(cfg.NKT):
            for half in range(2):
                t = xst.tile([P, XS], bf, name='xst', tag='xst')
                nc.sync.dma_start(t[:, :], xkT[kt * P:(kt + 1) * P, half * XS:(half + 1) * XS])
                xk_t.append(t)  # index 2*kt+half

        def kp_block(dt_):
            for sfp in range(S_ // (2 * FS)):
                ps = psA.tile([P, 2 * FS], f32, name='psa', tag='psa')
                for kt in range(cfg.NKT):
                    wt = wsm.tile([P, P], bf, name='wsm', tag='wsm')
                    nc.sync.dma_start(wt[:, :], wk[kt * P:(kt + 1) * P, dt_ * P:(dt_ + 1) * P])
                    for g in range(2):
                        sfg = 2 * sfp + g
                        half, sf = sfg // (XS // FS), sfg % (XS // FS)
                        nc.tensor.matmul(
                            ps[:, g * FS:(g + 1) * FS], wt[:, :],
                            xk_t[2 * kt + half][:, sf * FS:(sf + 1) * FS],
                            start=(kt == 0), stop=False)
                for g in range(2):
                    nc.tensor.matmul(
                        ps[:, g * FS:(g + 1) * FS],
                        bias_sb["bk"][0:1, dt_ * P:(dt_ + 1) * P],
                        ones[0:1, :], start=False, stop=True)
                nc.scalar.activation(
                    kpt_t[dt_][:, 2 * sfp * FS:(2 * sfp + 2) * FS], ps[:, :],
                    mybir.ActivationFunctionType.Copy, bias=0.0)

        def att_block(hp):
            # attention for this head pair on PE row-strips; both query
            # slices interleaved so PSUM accumulation never drains the pipe
            h0, h1 = 2 * hp, 2 * hp + 1
            po = {(qf, hh): psO.tile([DK + 1, FS], f32, name='pso', tag='pso')
                  for qf in range(cfg.NQF) for hh in (h0, h1)}
            for rt in range(cfg.NSP):
                for qf in range(cfg.NQF):
                    tsp = psA.tile([P, 2 * FS], f32, name='psa', tag='psa')
                    nc.tensor.matmul(
                        tsp[:, 0:FS], kpt_t[hp][0:DK, rt * P:(rt + 1) * P],
                        qpt_t[hp][0:DK, qf * FS:(qf + 1) * FS], start=True, stop=True)
                    nc.tensor.matmul(
                        tsp[:, FS:2 * FS], kpt_t[hp][DK:P, rt * P:(rt + 1) * P],
                        qpt_t[hp][DK:P, qf * FS:(qf + 1) * FS], start=True, stop=True)
                    pp = pex.tile([P, 2 * FS], bf, name='pex', tag='pex')
                    nc.scalar.activation(pp[:, :], tsp[:, :],
                                         mybir.ActivationFunctionType.Exp)
                    nc.tensor.matmul(
                        po[(qf, h0)][:, :], vpr_t[rt][:, h0 * VW:h0 * VW + VW],
                        pp[:, 0:FS],
                        start=(rt == 0), stop=(rt == cfg.NSP - 1))
                    nc.tensor.matmul(
                        po[(qf, h1)][:, :], vpr_t[rt][:, h1 * VW:h1 * VW + VW],
                        pp[:, FS:2 * FS],
                        start=(rt == 0), stop=(rt == cfg.NSP - 1))
            poss = {}
            for qf in range(cfg.NQF):
                for hh in (h0, h1):
                    # free the PSUM banks fast (before any slow reciprocal)
                    pos = posp.tile([DK + 1, FS], f32, name='pos', tag='pos')
                    nc.vector.tensor_copy(pos[:, :], po[(qf, hh)][:, :])
                    poss[(qf, hh)] = pos
            for qf in range(cfg.NQF):
                for hh in (h0, h1):
                    pos = poss[(qf, hh)]
                    rc = recp.tile([DK + 1, FS], f32, name='rec', tag='rec')
                    nc.vector.reciprocal(rc[DK:DK + 1, :], pos[DK:DK + 1, :])
                    # broadcast the recip row across DK partitions via a
                    # DRAM bounce (stride-0 partition read)
                    den_d = dram.tile([1, FS], f32, name='den_d', tag='den_d', bufs=4)
                    nc.sync.dma_start(den_d[:, :], rc[DK:DK + 1, :])
                    d6 = d64p.tile([DK, FS], f32, name='d64', tag='d64')
                    nc.gpsimd.dma_start(d6[:, :], den_d.broadcast_to([DK, FS]))
                    ot = otc.tile([DK, FS], bf, name='otc', tag='otc')
                    nc.vector.tensor_mul(ot[:, :], pos[0:DK, :], d6[:, :])
                    nc.sync.dma_start(
                        ot_d[qf][hh * DK:(hh + 1) * DK, :], ot[:, :])

        for hp in range(cfg.NHP):
            kp_block(hp)
            att_block(hp)

        # --- output projection: out[rq, df] = O @ wo + bo ---
        for df in range(cfg.NDF):
            wo_t = []
            for kt in range(cfg.NKT):
                t = wbg.tile([P, FS], bf, name='wbg', tag='wbg')
                nc.sync.dma_start(t[:, :], wo[kt * P:(kt + 1) * P, df * FS:(df + 1) * FS])
                wo_t.append(t)
            for rqt in range(SQ // P):
                ps = psA.tile([P, FS], f32, name='psa', tag='psa')
                for kt in range(cfg.NKT):
                    ot_t = wsm.tile([P, P], bf, name='wsm', tag='wsm')
                    rqf, rql = divmod(rqt, FS // P)
                    nc.sync.dma_start(ot_t[:, :], ot_d[rqf][kt * P:(kt + 1) * P, rql * P:(rql + 1) * P])
                    nc.tensor.matmul(ps[:, :], ot_t[:, :], wo_t[kt][:, :],
                                     start=(kt == 0), stop=False)
                nc.tensor.matmul(
                    ps[:, :], ones[0:1, 0:P],
                    bias_sb["bo"][0:1, df * FS:(df + 1) * FS],
                    start=False, stop=True)
                ct = cps.tile([P, FS], f32, name='cps', tag='cps')
                nc.vector.tensor_copy(ct[:, :], ps[:, :])
                nc.sync.dma_start(out[rqt * P:(rqt + 1) * P, df * FS:(df + 1) * FS], ct[:, :])

    nc.compile()
    return nc


def shard_inputs(q, k, v, wq, bq, wk, bk, wv, bv, wo, bo, sq, fs):
    """Build the 8 per-core input maps (bf16 operands) for the full problem."""
    import ml_dtypes
    bf = ml_dtypes.bfloat16
    scale = np.float32(1.0 / np.sqrt(DK))

    def c(x, dt=bf):
        return np.ascontiguousarray(np.asarray(x, np.float32).astype(dt))

    common = {
        "wq": c(np.asarray(wq, np.float32) * scale),
        "wk": c(wk), "wv": c(wv), "wo": c(wo),
        "bq": c((np.asarray(bq, np.float32) * scale).reshape(1, -1)),
        "bk": c(np.asarray(bk, np.float32).reshape(1, -1)),
        "bv": c(np.asarray(bv, np.float32).reshape(1, -1)),
        "bo": c(np.asarray(bo, np.float32).reshape(1, -1)),
        "onesb": np.ones((P, fs), bf),
    }
    q = np.asarray(q, np.float32)
    k = np.asarray(k, np.float32)
    v = np.asarray(v, np.float32)
    in_maps = []
    for core in range(NCORES):
        b, hf = core // 2, core % 2
        m = dict(common)
        m["xqT"] = c(q[b, hf * sq:(hf + 1) * sq, :].T)
        m["xkT"] = c(k[b].T)
        m["xvT"] = c(v[b].T)
        in_maps.append(m)
    return in_maps


_cached = {}


def kernel(q, k, v, wq, bq, wk, bk, wv, bv, wo, bo):
    from concourse import bass_utils

    global LAST_RESULTS
    cfg = Cfg(*FULL_CFG_ARGS)
    if "nc" not in _cached:
        _cached["nc"] = build_nc(cfg)
    nc = _cached["nc"]
    in_maps = shard_inputs(q, k, v, wq, bq, wk, bk, wv, bv, wo, bo, cfg.SQ, cfg.FS)
    res = bass_utils.run_bass_kernel_spmd(
        nc, in_maps, core_ids=list(range(NCORES)), trace=TRACE)
    LAST_RESULTS["res"] = res
    out = np.empty((B, S, D), np.float32)
    for core in range(NCORES):
        b, hf = core // 2, core % 2
        out[b, hf * cfg.SQ:(hf + 1) * cfg.SQ, :] = res.results[core]["out"]
    return out


# revision 24
# speedup vs baseline: 1.3620x; 1.0756x over previous
# Multi-head attention (B=4, S=2048, D=1024, H=16) on 8 TRN2 NeuronCores.
#
# Sharding: 8 cores = 4 batches x 2 query-halves. Each core:
#   - projects K/V for its batch's full sequence (duplicated across the 2
#     cores that share a batch -- cheaper than any collective),
#   - projects Q for its 1024 query rows,
#   - runs all 16 heads of attention for those query rows,
#   - computes the final output projection for its rows.
# Host-side work is only slicing/transposing/bf16-casting inputs and
# concatenating the 8 output slices -- no reductions happen on the host.
#
# Kernel-internal choices:
#   - activation inputs are pre-transposed on host to [D, rows] so every
#     matmul contracts over the partition dim,
#   - bf16 operands with fp32 PSUM accumulation (halves SBUF + DMA and
#     enables fast weight loads); the softmax denominator reciprocal path
#     stays fp32/f32r,
#   - Q/K/V projections are SBUF-resident (no DRAM roundtrips); only the
#     per-head attention output bounces through DRAM for the final matmul,
#   - softmax skips the max-subtraction (scores ~ N(0,1) by construction;
#     the 1/sqrt(dk) scale is folded into wq on the host),
#   - the softmax denominator comes from a ones-column appended to each
#     V tile, so it lands in the same PSUM tile as the attn@V output,
#   - scores for a head pair run on PE row-strips (partitions 0-63/64-127)
#     so the two dk=64 matmuls overlap on the systolic array,
#   - phase order QP -> VP -> KP: attention consumes per-tile deps on the
#     resident K tiles, so it overlaps the tail of the K projection.

import numpy as np

B, S, D, H, DK = 4, 2048, 1024, 16, 64
P = 128
NCORES = 8

TRACE = False  # set by test.py to capture an NTFF profile
LAST_RESULTS = {}  # test.py reads exec_time_ns etc. from here


class Cfg:
    def __init__(self, D_, S_, SQ, H_, FS):
        assert D_ == H_ * DK
        self.D, self.S, self.SQ, self.H, self.FS = D_, S_, SQ, H_, FS
        self.XS = S_ // 2          # x-stage slice width (half of kv seq)
        self.ND = D_ // P          # d_model partition tiles
        self.NKT = D_ // P         # contraction tiles over d_model
        self.NQF = SQ // FS        # query free-dim slices
        self.NSP = S_ // P         # key partition tiles
        self.NHP = H_ // 2         # head pairs
        self.NDF = D_ // FS        # d_model free-dim slices
        self.HPF = FS // DK        # heads per FS slice
        assert self.XS % FS == 0 and self.XS % P == 0
        assert SQ % P == 0 and SQ <= self.XS * 2
        assert H_ % 2 == 0 and FS % DK == 0


FULL_CFG_ARGS = (D, S, S // 2, H, 512)


def build_nc(cfg: Cfg):
    import concourse.mybir as mybir
    import concourse.tile as tile
    from concourse import bacc
    from contextlib import ExitStack

    f32 = mybir.dt.float32
    f32r = mybir.dt.float32r
    bf = mybir.dt.bfloat16

    D_, S_, SQ, FS, XS = cfg.D, cfg.S, cfg.SQ, cfg.FS, cfg.XS
    VW = DK + 1  # v-tile width incl. ones column

    nc = bacc.Bacc("TRN2", debug=False, num_devices=NCORES)

    xqT = nc.dram_tensor("xqT", [D_, SQ], bf, kind="ExternalInput").ap()
    xkT = nc.dram_tensor("xkT", [D_, S_], bf, kind="ExternalInput").ap()
    xvT = nc.dram_tensor("xvT", [D_, S_], bf, kind="ExternalInput").ap()
    wq = nc.dram_tensor("wq", [D_, D_], bf, kind="ExternalInput").ap()
    wk = nc.dram_tensor("wk", [D_, D_], bf, kind="ExternalInput").ap()
    wv = nc.dram_tensor("wv", [D_, D_], bf, kind="ExternalInput").ap()
    wo = nc.dram_tensor("wo", [D_, D_], bf, kind="ExternalInput").ap()
    bq = nc.dram_tensor("bq", [1, D_], bf, kind="ExternalInput").ap()
    bk = nc.dram_tensor("bk", [1, D_], bf, kind="ExternalInput").ap()
    bv = nc.dram_tensor("bv", [1, D_], bf, kind="ExternalInput").ap()
    bo = nc.dram_tensor("bo", [1, D_], bf, kind="ExternalInput").ap()
    onesb = nc.dram_tensor("onesb", [P, FS], bf, kind="ExternalInput").ap()
    out = nc.dram_tensor("out", [SQ, D_], f32, kind="ExternalOutput").ap()

    with tile.TileContext(nc) as tc, ExitStack() as ctx:
        pool = lambda name, bufs, space=None: ctx.enter_context(
            tc.tile_pool(name=name, bufs=bufs, **({"space": space} if space else {}))
        )
        kptp = pool("kpt", cfg.ND)     # resident K^T  [P, S] bf16
        qptp = pool("qpt", cfg.ND)     # resident Q^T  [P, SQ] bf16
        vprp = pool("vpr", cfg.NSP)    # resident V    [P, H*VW] bf16
        xst = pool("xst", 16)
        xqp = pool("xqp", 8)
        wsm = pool("wsm", 20)
        wbg = pool("wbg", 18)
        pex = pool("pex", 10)
        otc = pool("otc", 3)
        recp = pool("rec", 2)
        posp = pool("pos", 5)
        d64p = pool("d64", 2)
        cps = pool("cps", 3)
        cst = pool("cst", 1)
        psA = pool("psA", 2, "PSUM")
        psO = pool("psO", 4, "PSUM")
        dram = pool("dram", 1, "DRAM")

        ot_d = [dram.tile([D_, FS], bf, name="ot_d", tag=f"ot_d{i}")
                for i in range(cfg.NQF)]

        # --- constants ---
        bias_sb = {}
        for name, ap in (("bq", bq), ("bk", bk), ("bv", bv), ("bo", bo)):
            t = cst.tile([1, D_], bf, name=name, tag=name)
            nc.sync.dma_start(t[:, :], ap[:, :])
            bias_sb[name] = t
        ones = cst.tile([P, FS], bf, name="ones", tag="ones")
        nc.sync.dma_start(ones[:, :], onesb[:, :])

        # resident tiles
        qpt_t = [qptp.tile([P, SQ], bf, name='qptr', tag='qptr') for _ in range(cfg.ND)]
        kpt_t = [kptp.tile([P, S_], bf, name='kpt', tag='kpt') for _ in range(cfg.ND)]
        vpr_t = [vprp.tile([P, cfg.H * VW], bf, name='vpr', tag='vpr')
                 for _ in range(cfg.NSP)]

        # --- Q projection: QPT[d_out, rq] = (xq @ wq + bq)^T, resident ---
        xq_t = []
        for kt in range(cfg.NKT):
            t = xqp.tile([P, SQ], bf, name='xqp', tag='xqp')
            nc.sync.dma_start(t[:, :], xqT[kt * P:(kt + 1) * P, :])
            xq_t.append(t)
        xv_t = []
        for kt in range(cfg.NKT):
            for half in range(2):
                t = xst.tile([P, XS], bf, name='xst', tag='xst')
                nc.sync.dma_start(t[:, :], xvT[kt * P:(kt + 1) * P, half * XS:(half + 1) * XS])
                xv_t.append(t)  # index 2*kt+half
        for dt_ in range(cfg.ND):
            ps = psA.tile([P, cfg.NQF * FS], f32, name='psa', tag='psa')
            for kt in range(cfg.NKT):
                wt = wsm.tile([P, P], bf, name='wsm', tag='wsm')
                nc.sync.dma_start(wt[:, :], wq[kt * P:(kt + 1) * P, dt_ * P:(dt_ + 1) * P])
                for qf in range(cfg.NQF):
                    nc.tensor.matmul(
                        ps[:, qf * FS:(qf + 1) * FS], wt[:, :],
                        xq_t[kt][:, qf * FS:(qf + 1) * FS],
                        start=(kt == 0), stop=False)
            for qf in range(cfg.NQF):
                nc.tensor.matmul(
                    ps[:, qf * FS:(qf + 1) * FS],
                    bias_sb["bq"][0:1, dt_ * P:(dt_ + 1) * P],
                    ones[0:1, :], start=False, stop=True)
            nc.vector.tensor_copy(qpt_t[dt_][:, :], ps[:, 0:SQ])

        # --- V projection: VP[rk, dv] head-major, resident w/ ones columns ---
        for df in range(cfg.NDF):
            wv_t = []
            for kt in range(cfg.NKT):
                t = wbg.tile([P, FS], bf, name='wbg', tag='wbg')
                nc.sync.dma_start(t[:, :], wv[kt * P:(kt + 1) * P, df * FS:(df + 1) * FS])
                wv_t.append(t)
            for rk in range(cfg.NSP):
                half, rt = rk // (XS // P), rk % (XS // P)
                ps = psA.tile([P, FS], f32, name='psa', tag='psa')
                for kt in range(cfg.NKT):
                    nc.tensor.matmul(
                        ps[:, :], xv_t[2 * kt + half][:, rt * P:(rt + 1) * P],
                        wv_t[kt][:, :], start=(kt == 0), stop=False)
                nc.tensor.matmul(
                    ps[:, :], ones[0:1, 0:P],
                    bias_sb["bv"][0:1, df * FS:(df + 1) * FS],
                    start=False, stop=True)
                h0 = df * cfg.HPF
                dst = vpr_t[rk].rearrange("p (h c) -> p h c", c=VW)
                nc.vector.tensor_copy(
                    dst[:, h0:h0 + cfg.HPF, 0:DK],
                    ps.rearrange("p (h v) -> p h v", v=DK))
        for rk in range(cfg.NSP):
            dst = vpr_t[rk].rearrange("p (h c) -> p h c", c=VW)
            nc.vector.memset(dst[:, :, DK:DK + 1], 1.0)

        # --- K projection interleaved with attention: KP(dt) feeds ATT(hp=dt),
        # so PE-heavy projection work fills the ACT-bound attention slack ---
        xk_t = []
        for kt in range(cfg.NKT):
            for half in range(2):
                t = xst.tile([P, XS], bf, name='xst', tag='xst')
                nc.sync.dma_start(t[:, :], xkT[kt * P:(kt + 1) * P, half * XS:(half + 1) * XS])
                xk_t.append(t)  # index 2*kt+half

        def kp_block(dt_):
            for sfp in range(S_ // (2 * FS)):
                ps = psA.tile([P, 2 * FS], f32, name='psa', tag='psa')
                for kt in range(cfg.NKT):
                    wt = wsm.tile([P, P], bf, name='wsm', tag='wsm')
                    nc.sync.dma_start(wt[:, :], wk[kt * P:(kt + 1) * P, dt_ * P:(dt_ + 1) * P])
                    for g in range(2):
                        sfg = 2 * sfp + g
                        half, sf = sfg // (XS // FS), sfg % (XS // FS)
                        nc.tensor.matmul(
                            ps[:, g * FS:(g + 1) * FS], wt[:, :],
                            xk_t[2 * kt + half][:, sf * FS:(sf + 1) * FS],
                            start=(kt == 0), stop=False)
                for g in range(2):
                    nc.tensor.matmul(
                        ps[:, g * FS:(g + 1) * FS],
                        bias_sb["bk"][0:1, dt_ * P:(dt_ + 1) * P],
                        ones[0:1, :], start=False, stop=True)
                nc.scalar.activation(
                    kpt_t[dt_][:, 2 * sfp * FS:(2 * sfp + 2) * FS], ps[:, :],
                    mybir.ActivationFunctionType.Copy, bias=0.0)

        def att_block(hp):
            # attention for this head pair on PE row-strips; both query
            # slices interleaved so PSUM accumulation never drains the pipe
            h0, h1 = 2 * hp, 2 * hp + 1
            po = {(qf, hh): psO.tile([DK + 1, FS], f32, name='pso', tag='pso')
                  for qf in range(cfg.NQF) for hh in (h0, h1)}
            for rt in range(cfg.NSP):
                for qf in range(cfg.NQF):
                    tsp = psA.tile([P, 2 * FS], f32, name='psa', tag='psa')
                    nc.tensor.matmul(
                        tsp[:, 0:FS], kpt_t[hp][0:DK, rt * P:(rt + 1) * P],
                        qpt_t[hp][0:DK, qf * FS:(qf + 1) * FS], start=True, stop=True)
                    nc.tensor.matmul(
                        tsp[:, FS:2 * FS], kpt_t[hp][DK:P, rt * P:(rt + 1) * P],
                        qpt_t[hp][DK:P, qf * FS:(qf + 1) * FS], start=True, stop=True)
                    pp = pex.tile([P, 2 * FS], bf, name='pex', tag='pex')
                    nc.scalar.activation(pp[:, :], tsp[:, :],
                                         mybir.ActivationFunctionType.Exp)
                    nc.tensor.matmul(
                        po[(qf, h0)][:, :], vpr_t[rt][:, h0 * VW:h0 * VW + VW],
                        pp[:, 0:FS],
                        start=(rt == 0), stop=(rt == cfg.NSP - 1))
                    nc.tensor.matmul(
                        po[(qf, h1)][:, :], vpr_t[rt][:, h1 * VW:h1 * VW + VW],
                        pp[:, FS:2 * FS],
                        start=(rt == 0), stop=(rt == cfg.NSP - 1))
            poss = {}
            for qf in range(cfg.NQF):
                for hh in (h0, h1):
                    # free the PSUM banks fast (before any slow reciprocal)
                    pos = posp.tile([DK + 1, FS], f32, name='pos', tag='pos')
                    nc.vector.tensor_copy(pos[:, :], po[(qf, hh)][:, :])
                    poss[(qf, hh)] = pos
            for qf in range(cfg.NQF):
                for hh in (h0, h1):
                    pos = poss[(qf, hh)]
                    rc = recp.tile([DK + 1, FS], f32, name='rec', tag='rec')
                    nc.vector.reciprocal(rc[DK:DK + 1, :], pos[DK:DK + 1, :])
                    # broadcast the recip row across DK partitions via a
                    # DRAM bounce (stride-0 partition read)
                    den_d = dram.tile([1, FS], f32, name='den_d', tag='den_d', bufs=4)
                    nc.sync.dma_start(den_d[:, :], rc[DK:DK + 1, :])
                    d6 = d64p.tile([DK, FS], f32, name='d64', tag='d64')
                    nc.gpsimd.dma_start(d6[:, :], den_d.broadcast_to([DK, FS]))
                    ot = otc.tile([DK, FS], bf, name='otc', tag='otc')
                    nc.vector.tensor_mul(ot[:, :], pos[0:DK, :], d6[:, :])
                    nc.sync.dma_start(
                        ot_d[qf][hh * DK:(hh + 1) * DK, :], ot[:, :])

        for hp in range(cfg.NHP):
            kp_block(hp)
            att_block(hp)

        # --- output projection: out[rq, df] = O @ wo + bo ---
        # load each ot_d half once as wide tiles; lhsT slices come from SBUF
        for qf in range(cfg.NQF):
            ot8 = []
            for kt in range(cfg.NKT):
                t = wbg.tile([P, FS], bf, name='wbg', tag='wbg')
                nc.sync.dma_start(t[:, :], ot_d[qf][kt * P:(kt + 1) * P, :])
                ot8.append(t)
            for df in range(cfg.NDF):
                wo_t = []
                for kt in range(cfg.NKT):
                    t = wbg.tile([P, FS], bf, name='wbg', tag='wbg')
                    nc.sync.dma_start(t[:, :], wo[kt * P:(kt + 1) * P, df * FS:(df + 1) * FS])
                    wo_t.append(t)
                for rql in range(FS // P):
                    rqt = qf * (FS // P) + rql
                    ps = psA.tile([P, FS], f32, name='psa', tag='psa')
                    for kt in range(cfg.NKT):
                        nc.tensor.matmul(
                            ps[:, :], ot8[kt][:, rql * P:(rql + 1) * P],
                            wo_t[kt][:, :], start=(kt == 0), stop=False)
                    nc.tensor.matmul(
                        ps[:, :], ones[0:1, 0:P],
                        bias_sb["bo"][0:1, df * FS:(df + 1) * FS],
                        start=False, stop=True)
                    ct = cps.tile([P, FS], f32, name='cps', tag='cps')
                    nc.vector.tensor_copy(ct[:, :], ps[:, :])
                    nc.sync.dma_start(out[rqt * P:(rqt + 1) * P, df * FS:(df + 1) * FS], ct[:, :])

    nc.compile()
    return nc


def shard_inputs(q, k, v, wq, bq, wk, bk, wv, bv, wo, bo, sq, fs):
    """Build the 8 per-core input maps (bf16 operands) for the full problem."""
    import ml_dtypes
    bf = ml_dtypes.bfloat16
    scale = np.float32(1.0 / np.sqrt(DK))

    def c(x, dt=bf):
        return np.ascontiguousarray(np.asarray(x, np.float32).astype(dt))

    common = {
        "wq": c(np.asarray(wq, np.float32) * scale),
        "wk": c(wk), "wv": c(wv), "wo": c(wo),
        "bq": c((np.asarray(bq, np.float32) * scale).reshape(1, -1)),
        "bk": c(np.asarray(bk, np.float32).reshape(1, -1)),
        "bv": c(np.asarray(bv, np.float32).reshape(1, -1)),
        "bo": c(np.asarray(bo, np.float32).reshape(1, -1)),
        "onesb": np.ones((P, fs), bf),
    }
    q = np.asarray(q, np.float32)
    k = np.asarray(k, np.float32)
    v = np.asarray(v, np.float32)
    in_maps = []
    for core in range(NCORES):
        b, hf = core // 2, core % 2
        m = dict(common)
        m["xqT"] = c(q[b, hf * sq:(hf + 1) * sq, :].T)
        m["xkT"] = c(k[b].T)
        m["xvT"] = c(v[b].T)
        in_maps.append(m)
    return in_maps


_cached = {}


def kernel(q, k, v, wq, bq, wk, bk, wv, bv, wo, bo):
    from concourse import bass_utils

    global LAST_RESULTS
    cfg = Cfg(*FULL_CFG_ARGS)
    if "nc" not in _cached:
        _cached["nc"] = build_nc(cfg)
    nc = _cached["nc"]
    in_maps = shard_inputs(q, k, v, wq, bq, wk, bk, wv, bv, wo, bo, cfg.SQ, cfg.FS)
    res = bass_utils.run_bass_kernel_spmd(
        nc, in_maps, core_ids=list(range(NCORES)), trace=TRACE)
    LAST_RESULTS["res"] = res
    out = np.empty((B, S, D), np.float32)
    for core in range(NCORES):
        b, hf = core // 2, core % 2
        out[b, hf * cfg.SQ:(hf + 1) * cfg.SQ, :] = res.results[core]["out"]
    return out
